# revision 1
# baseline (speedup 1.0000x reference)
"""Invariant Point Attention kernel for Trainium2, 8-core SPMD.

Strategy: sequence-parallel over the query axis n (96 rows/core). Each core
computes full k/v/k_pts from `single` (replicated, tiny), its own q rows, its
own [96, 768] slice of the pair tensor (host-transposed to [PC, n, m] so the
PC=128 contraction lands on SBUF partitions), full attention rows (softmax
over m is core-local -> zero collectives), and its [96, 384] output slice.

Math notes vs the reference:
  - terms constant along the softmax axis m cancel exactly (q2, bk, bpb) and
    are dropped;
  - SCALE is folded into Wq/bq, Wqp/bqp and the q-side trans on the host;
  - softmax runs without max-subtraction (logits are O(10), exp is safe in
    fp32); the denominator is applied after the attn@v matmul by linearity.
"""

import sys

for p in ("/opt/trn_rl_repo", "/opt/trn_rl_repo/concourse"):
    if p not in sys.path:
        sys.path.append(p)

import numpy as np

import concourse.bass as bass
import concourse.tile as tile
from concourse import bacc, mybir
from concourse.bass_utils import run_bass_kernel_spmd

F32 = mybir.dt.float32
AX = mybir.AxisListType
ALU = mybir.AluOpType
ACTF = mybir.ActivationFunctionType

B, N, C, PC, H, P = 1, 768, 384, 128, 12, 3
Ch = C // H            # 32
HD = H * P * P         # 108
SCALE = Ch ** -0.5
EPS = 1e-5
NCORES = 8
NO = N // NCORES       # 96 own query rows per core
NG = 4                 # n rows per pair-phase group
GROUPS = NO // NG      # 12
MT = N // 128          # 6 m tiles


def _build(nc):
    dt_ = lambda name, shape: nc.dram_tensor(name, shape, F32, kind="ExternalInput").ap()
    singleT = dt_("singleT", [C, N])
    sTo = dt_("sTo", [C, NO])
    so = dt_("so", [NO, C])
    pairT = dt_("pairT", [PC, NO, N])
    rot9 = dt_("rot9", [N, 9])
    roto = dt_("roto", [NO, 9])
    trans3 = dt_("trans3", [N, 3])
    transqo = dt_("transqo", [NO, 3])
    Wq = dt_("Wq", [C, 512])
    bq = dt_("bq", [1, 512])
    Wk = dt_("Wk", [C, 512])
    Wv = dt_("Wv", [C, C])
    bv = dt_("bv", [1, C])
    Wqp = dt_("Wqp", [C, HD])
    bqp = dt_("bqp", [1, HD])
    Wkp = dt_("Wkp", [C, HD])
    bkp = dt_("bkp", [1, HD])
    Wpb = dt_("Wpb", [PC, H])
    Wo = dt_("Wo", [C, C])
    bo = dt_("bo", [1, C])
    gamB = dt_("gamB", [NO, C])
    betB = dt_("betB", [NO, C])
    id128 = dt_("id128", [128, 128])
    out = nc.dram_tensor("out", [NO, C], F32, kind="ExternalOutput").ap()

    with tile.TileContext(nc) as tc:
        _kernel(tc, locals())
    return out


def _kernel(tc, t):
    nc = tc.nc
    mm = nc.tensor.matmul
    dma = nc.sync.dma_start

    const = tc.alloc_tile_pool(name="const", bufs=1)
    big = tc.alloc_tile_pool(name="big", bufs=1)

    # ---- load constants / weights ----
    def load(name, shape, src):
        tl = const.tile(list(shape), F32, tag=name)
        dma(tl[:], src)
        return tl

    Wq_sb = const.tile([128, 3 * 512], F32, tag="Wq_sb")
    Wk_sb = const.tile([128, 3 * 512], F32, tag="Wk_sb")
    for W_sb, name in ((Wq_sb, "Wq"), (Wk_sb, "Wk")):
        for tt in range(3):
            dma(W_sb[:, tt * 512:(tt + 1) * 512], t[name][tt * 128:(tt + 1) * 128, :])
    Wv_sb = const.tile([128, 3 * C], F32, tag="Wv_sb")
    Wo_sb = const.tile([128, 3 * C], F32, tag="Wo_sb")
    for W_sb, name in ((Wv_sb, "Wv"), (Wo_sb, "Wo")):
        for tt in range(3):
            dma(W_sb[:, tt * C:(tt + 1) * C], t[name][tt * 128:(tt + 1) * 128, :])
    Wqp_sb = const.tile([128, 3 * HD], F32, tag="Wqp_sb")
    Wkp_sb = const.tile([128, 3 * HD], F32, tag="Wkp_sb")
    for W_sb, name in ((Wqp_sb, "Wqp"), (Wkp_sb, "Wkp")):
        for tt in range(3):
            dma(W_sb[:, tt * HD:(tt + 1) * HD], t[name][tt * 128:(tt + 1) * 128, :])
    Wpb_sb = load("Wpb_sb", (PC, H), t["Wpb"])
    bq_sb = load("bq_sb", (1, 512), t["bq"])
    bv_sb = load("bv_sb", (1, C), t["bv"])
    bqp_sb = load("bqp_sb", (1, HD), t["bqp"])
    bkp_sb = load("bkp_sb", (1, HD), t["bkp"])
    bo_sb = load("bo_sb", (1, C), t["bo"])
    id_sb = load("id_sb", (128, 128), t["id128"])
    gam_sb = load("gam_sb", (NO, C), t["gamB"])
    bet_sb = load("bet_sb", (NO, C), t["betB"])
    so_sb = load("so_sb", (NO, C), t["so"])
    roto_sb = load("roto_sb", (NO, 9), t["roto"])
    transqo_sb = load("transqo_sb", (NO, 3), t["transqo"])
    sT_sb = const.tile([128, 3 * N], F32, tag="sT_sb")
    for tt in range(3):
        dma(sT_sb[:, tt * N:(tt + 1) * N], t["singleT"][tt * 128:(tt + 1) * 128, :])
    sTo_sb = const.tile([128, 3 * NO], F32, tag="sTo_sb")
    for tt in range(3):
        dma(sTo_sb[:, tt * NO:(tt + 1) * NO], t["sTo"][tt * 128:(tt + 1) * 128, :])
    rot_sb = const.tile([128, 6 * 9], F32, tag="rot_sb")
    trans_sb = const.tile([128, 6 * 3], F32, tag="trans_sb")
    for mt in range(MT):
        dma(rot_sb[:, mt * 9:(mt + 1) * 9], t["rot9"][mt * 128:(mt + 1) * 128, :])
        dma(trans_sb[:, mt * 3:(mt + 1) * 3], t["trans3"][mt * 128:(mt + 1) * 128, :])
    ones_col = const.tile([128, 1], F32, tag="ones_col")
    nc.vector.memset(ones_col[:], 1.0)
    ones96 = const.tile([1, NO], F32, tag="ones96")
    nc.vector.memset(ones96[:], 1.0)
    ones128 = const.tile([1, 128], F32, tag="ones128")
    nc.vector.memset(ones128[:], 1.0)

    # ---- big persistent sbuf ----
    kT_sb = big.tile([128, 4 * N], F32, tag="kT")        # [c_out, m] (3 row-tiles)
    qT_sb = big.tile([128, 4 * NO], F32, tag="qT")       # [c_out, n]
    v_sb = big.tile([128, MT * C], F32, tag="v")         # per m-tile [128, 384]
    qg_sb = big.tile([NO, HD], F32, tag="qg")
    kg_sb = big.tile([128, MT * HD], F32, tag="kg")
    # per-head transposed points, head h at partitions 32*(h%4), col block h//4
    qgT_sb = big.tile([128, 4 * NO], F32, tag="qgT")
    kgT_sb = big.tile([128, 4 * N], F32, tag="kgT")
    k2s_sb = big.tile([128, MT * H], F32, tag="k2s")     # -0.5*SCALE*k2, per m-tile
    pb_sb = big.tile([128, MT * H * NO], F32, tag="pb")  # pair bias [m | (h, n)]
    E_sb = big.tile([128, MT * H * NO], F32, tag="E")    # exp(logits) [m | (h, n)]

    with tc.tile_pool(name="pro", bufs=3, space="PSUM") as pro, \
         tc.tile_pool(name="work", bufs=6) as work:

        # ---- kT = (single @ Wk)^T : [c_out, m], no bias (cancels in softmax)
        for j in range(4):
            for half in range(2):
                ps = pro.tile([128, 384], F32, tag="ps")
                for tt in range(3):
                    mm(ps[:], Wk_sb[:, tt * 512 + j * 128: tt * 512 + (j + 1) * 128],
                       sT_sb[:, tt * N + half * 384: tt * N + (half + 1) * 384],
                       start=(tt == 0), stop=(tt == 2))
                nc.vector.tensor_copy(kT_sb[:, j * N + half * 384: j * N + (half + 1) * 384], ps[:])

        # ---- qT = (single_own @ (SCALE*Wq))^T + SCALE*bq : [c_out, n]
        for j in range(4):
            ps = pro.tile([128, NO], F32, tag="ps")
            for tt in range(3):
                mm(ps[:], Wq_sb[:, tt * 512 + j * 128: tt * 512 + (j + 1) * 128],
                   sTo_sb[:, tt * NO:(tt + 1) * NO], start=(tt == 0), stop=False)
            mm(ps[:], bq_sb[0:1, j * 128:(j + 1) * 128], ones96[:], start=False, stop=True)
            nc.vector.tensor_copy(qT_sb[:, j * NO:(j + 1) * NO], ps[:])

        # ---- v = single @ Wv + bv : [m, c_out]
        for mt in range(MT):
            ps = pro.tile([128, 384], F32, tag="ps")
            for tt in range(3):
                mm(ps[:], sT_sb[:, tt * N + mt * 128: tt * N + (mt + 1) * 128],
                   Wv_sb[:, tt * C:(tt + 1) * C], start=(tt == 0), stop=False)
            mm(ps[:], ones128[:], bv_sb[:], start=False, stop=True)
            nc.vector.tensor_copy(v_sb[:, mt * C:(mt + 1) * C], ps[:])

        # ---- point projections: qp [n, 108] (SCALE folded), kp per m-tile
        qp_sb = work.tile([NO, HD], F32, tag="qp")
        ps = pro.tile([128, 384], F32, tag="ps")
        for tt in range(3):
            mm(ps[:NO, :HD], sTo_sb[:, tt * NO:(tt + 1) * NO],
               Wqp_sb[:, tt * HD:(tt + 1) * HD], start=(tt == 0), stop=False)
        mm(ps[:NO, :HD], ones96[:], bqp_sb[:], start=False, stop=True)
        nc.vector.tensor_copy(qp_sb[:], ps[:NO, :HD])

        kp_tiles = []
        for mt in range(MT):
            ps = pro.tile([128, 384], F32, tag="ps")
            for tt in range(3):
                mm(ps[:, :HD], sT_sb[:, tt * N + mt * 128: tt * N + (mt + 1) * 128],
                   Wkp_sb[:, tt * HD:(tt + 1) * HD], start=(tt == 0), stop=False)
            mm(ps[:, :HD], ones128[:], bkp_sb[:], start=False, stop=True)
            kp = work.tile([128, HD], F32, tag="kp")
            nc.vector.tensor_copy(kp[:], ps[:, :HD])
            kp_tiles.append(kp)

        # ---- rotations: g[n,h,d,j] = sum_i p[n,h,d,i]*rot[n,3i+j] (+ trans[n,d])
        def rotate(dst, src, rsb, roff, tsb, toff, rows):
            dv = dst.rearrange("p (h d j) -> p h d j", d=3, j=3)
            sv = src.rearrange("p (h d i) -> p h d i", d=3, i=3)
            for j in range(3):
                acc = work.tile([rows, 36], F32, tag="rotacc")
                av = acc[:].rearrange("p (h d) -> p h d", d=3)
                nc.vector.tensor_scalar_mul(av, sv[:, :, :, 0], rsb[:rows, roff + j: roff + j + 1])
                for i in (1, 2):
                    nc.vector.scalar_tensor_tensor(
                        av, sv[:, :, :, i], rsb[:rows, roff + 3 * i + j: roff + 3 * i + j + 1],
                        av, op0=ALU.mult, op1=ALU.add)
                nc.vector.tensor_copy(dv[:, :, :, j], av)
            for d in range(3):
                nc.vector.tensor_scalar_add(dv[:, :, d, :], dv[:, :, d, :],
                                            tsb[:rows, toff + d: toff + d + 1])

        rotate(qg_sb[:], qp_sb[:], roto_sb, 0, transqo_sb, 0, NO)
        for mt in range(MT):
            rotate(kg_sb[:, mt * HD:(mt + 1) * HD], kp_tiles[mt][:],
                   rot_sb, mt * 9, trans_sb, mt * 3, 128)

        # ---- k2s = -0.5*SCALE*sum_dj kg^2 : [m, h] per m-tile
        for mt in range(MT):
            sq = work.tile([128, HD], F32, tag="sq")
            kgs = kg_sb[:, mt * HD:(mt + 1) * HD]
            nc.vector.tensor_mul(sq[:], kgs, kgs)
            red = work.tile([128, H], F32, tag="red")
            nc.vector.tensor_reduce(red[:], sq[:].rearrange("p (h e) -> p h e", e=9),
                                    axis=AX.X, op=ALU.add)
            nc.vector.tensor_scalar_mul(k2s_sb[:, mt * H:(mt + 1) * H], red[:], -0.5 * SCALE)

        # ---- transpose qg, kg -> per-head [(d,j)=9 rows @ 32*(h%4), n/m]
        for h in range(H):
            bp, blk = 32 * (h % 3), h // 3
            ps = pro.tile([128, 384], F32, tag="ps")
            mm(ps[bp:bp + 9, :NO], qg_sb[:, h * 9:(h + 1) * 9],
               id_sb[:NO, :NO], start=True, stop=True)
            nc.vector.tensor_copy(qgT_sb[bp:bp + 9, blk * NO:(blk + 1) * NO],
                                  ps[bp:bp + 9, :NO])
        for mt in range(MT):
            for h in range(H):
                bp, blk = 32 * (h % 3), h // 3
                ps = pro.tile([128, 384], F32, tag="ps")
                mm(ps[bp:bp + 9, :128],
                   kg_sb[:, mt * HD + h * 9: mt * HD + (h + 1) * 9],
                   id_sb[:], start=True, stop=True)
                nc.vector.tensor_copy(
                    kgT_sb[bp:bp + 9, blk * N + mt * 128: blk * N + (mt + 1) * 128],
                    ps[bp:bp + 9, :128])

        # ---- pair phase: pb tile stationary -> out [m, h] directly
        with tc.tile_pool(name="pp", bufs=4, space="PSUM") as pp, \
             tc.tile_pool(name="pairp", bufs=2) as pairp:
            for g in range(GROUPS):
                pg = pairp.tile([128, NG * N], F32, tag="pg")
                dma(pg[:], t["pairT"][:, g * NG:(g + 1) * NG, :])
                for mt in range(MT):
                    ps = pp.tile([128, NG * H], F32, tag="ps")
                    for ns in range(NG):
                        mm(ps[:, ns * H:(ns + 1) * H],
                           pg[:, ns * N + mt * 128: ns * N + (mt + 1) * 128],
                           Wpb_sb[:], start=True, stop=True)
                    # ps is [m, (n8, h)]; scatter into pb_sb [m, (h, n)]
                    dst = pb_sb[:, mt * H * NO:(mt + 1) * H * NO] \
                        .rearrange("p (h n) -> p h n", h=H)[:, :, g * NG:(g + 1) * NG] \
                        .transpose([0, 2, 1])
                    nc.vector.tensor_copy(dst, ps[:].rearrange("p (n h) -> p n h", h=H))

    # ---- attention ----
    with tc.tile_pool(name="pL", bufs=4, space="PSUM") as pL, \
         tc.tile_pool(name="pacc", bufs=1, space="PSUM") as pacc, \
         tc.tile_pool(name="att", bufs=3) as att:
        av_ps = pacc.tile([NO, C], F32, tag="av")
        dn_ps = pacc.tile([NO, H], F32, tag="dn")
        for mt in range(MT):
            tmp = att.tile([128, H * NO], F32, tag="tmp")
            for h in range(H):
                L = pL.tile([128, NO], F32, tag="L")
                tl, tr = h // 3, 32 * (h % 3)
                mm(L[:], kT_sb[tr:tr + 32, tl * N + mt * 128: tl * N + (mt + 1) * 128],
                   qT_sb[tr:tr + 32, tl * NO:(tl + 1) * NO], start=True, stop=False)
                mm(L[:], kgT_sb[tr:tr + 9, tl * N + mt * 128: tl * N + (mt + 1) * 128],
                   qgT_sb[tr:tr + 9, tl * NO:(tl + 1) * NO], start=False, stop=True)
                nc.vector.scalar_tensor_tensor(
                    tmp[:, h * NO:(h + 1) * NO], L[:], k2s_sb[:, mt * H + h: mt * H + h + 1],
                    pb_sb[:, (mt * H + h) * NO:(mt * H + h + 1) * NO],
                    op0=ALU.add, op1=ALU.add)
            eslab = E_sb[:, mt * H * NO:(mt + 1) * H * NO]
            nc.scalar.activation(eslab, tmp[:], ACTF.Exp)
        for h in range(H):
            for mt in range(MT):
                e = E_sb[:, (mt * H + h) * NO:(mt * H + h + 1) * NO]
                mm(av_ps[:, h * Ch:(h + 1) * Ch], e,
                   v_sb[:, mt * C + h * Ch: mt * C + (h + 1) * Ch],
                   start=(mt == 0), stop=(mt == MT - 1))
            for mt in range(MT):
                e = E_sb[:, (mt * H + h) * NO:(mt * H + h + 1) * NO]
                mm(dn_ps[:, h:h + 1], e, ones_col[:], start=(mt == 0), stop=(mt == MT - 1))

        # ---- epilogue: divide, out-proj, residual, layernorm ----
        rcp = att.tile([NO, H], F32, tag="rcp")
        nc.vector.reciprocal(rcp[:], dn_ps[:])
        w_sb = att.tile([NO, C], F32, tag="w")
        for h in range(H):
            nc.vector.tensor_scalar_mul(w_sb[:, h * Ch:(h + 1) * Ch],
                                        av_ps[:, h * Ch:(h + 1) * Ch], rcp[:, h:h + 1])
        wT_sb = att.tile([128, 3 * NO], F32, tag="wT")
        for tt in range(3):
            tp = pL.tile([128, NO], F32, tag="L")
            nc.tensor.transpose(tp[:], w_sb[:, tt * 128:(tt + 1) * 128], id_sb[:NO, :NO])
            nc.vector.tensor_copy(wT_sb[:, tt * NO:(tt + 1) * NO], tp[:])
        o_ps = pacc.tile([NO, C], F32, tag="av")
        for tt in range(3):
            mm(o_ps[:], wT_sb[:, tt * NO:(tt + 1) * NO], Wo_sb[:, tt * C:(tt + 1) * C],
               start=(tt == 0), stop=False)
        mm(o_ps[:], ones96[:], bo_sb[:], start=False, stop=True)
        x_sb = att.tile([NO, C], F32, tag="x")
        nc.vector.tensor_add(x_sb[:], o_ps[:], so_sb[:])
        mu = att.tile([NO, 1], F32, tag="mu")
        nc.vector.tensor_reduce(mu[:], x_sb[:], axis=AX.X, op=ALU.add)
        nc.vector.tensor_scalar_mul(mu[:], mu[:], 1.0 / C)
        xm = att.tile([NO, C], F32, tag="xm")
        nc.vector.tensor_scalar_sub(xm[:], x_sb[:], mu[:])
        sq = att.tile([NO, C], F32, tag="sqe")
        nc.vector.tensor_mul(sq[:], xm[:], xm[:])
        var = att.tile([NO, 1], F32, tag="var")
        nc.vector.tensor_reduce(var[:], sq[:], axis=AX.X, op=ALU.add)
        epsb = att.tile([NO, 1], F32, tag="epsb")
        nc.vector.memset(epsb[:], EPS)
        std = att.tile([NO, 1], F32, tag="std")
        nc.scalar.activation(std[:], var[:], ACTF.Sqrt, bias=epsb[:], scale=1.0 / C)
        rstd = att.tile([NO, 1], F32, tag="rstd")
        nc.vector.reciprocal(rstd[:], std[:])
        y = att.tile([NO, C], F32, tag="y")
        nc.vector.tensor_scalar_mul(y[:], xm[:], rstd[:])
        nc.vector.tensor_mul(y[:], y[:], gam_sb[:])
        nc.vector.tensor_add(y[:], y[:], bet_sb[:])
        dma(t["out"], y[:])
    big.release()
    const.release()


_CACHE = {}


def _get_program():
    if "nc" not in _CACHE:
        nc = bacc.Bacc("TRN2", target_bir_lowering=False, debug=False,
                       num_devices=NCORES)
        _build(nc)
        nc.compile()
        _CACHE["nc"] = nc
    return _CACHE["nc"]


def _pad_heads(W):
    # scatter head h (32 cols) to col 128*(h//3) + 32*(h%3) of a 512-wide buffer
    out = np.zeros(W.shape[:-1] + (512,), np.float32)
    for h in range(H):
        out[..., 128 * (h // 3) + 32 * (h % 3): 128 * (h // 3) + 32 * (h % 3) + Ch] = \
            W[..., h * Ch:(h + 1) * Ch]
    return out


def make_in_maps(single, pair, rot, trans, Wq, bq, Wk, bk, Wv, bv, Wpb, bpb,
                 Wqp, bqp, Wkp, bkp, Wo, bo, gamma, beta):
    f = lambda a: np.ascontiguousarray(np.asarray(a), dtype=np.float32)
    s = f(single)[0]
    sT = f(s.T)
    common = {
        "singleT": sT,
        "rot9": f(rot)[0].reshape(N, 9),
        "trans3": f(trans)[0],
        "Wq": _pad_heads(f(Wq) * SCALE), "bq": _pad_heads((f(bq) * SCALE).reshape(1, C)),
        "Wk": _pad_heads(f(Wk)),
        "Wv": f(Wv), "bv": f(bv).reshape(1, C),
        "Wqp": f(Wqp) * SCALE, "bqp": (f(bqp) * SCALE).reshape(1, HD),
        "Wkp": f(Wkp), "bkp": f(bkp).reshape(1, HD),
        "Wpb": f(Wpb),
        "Wo": f(Wo), "bo": f(bo).reshape(1, C),
        "gamB": np.ascontiguousarray(np.broadcast_to(f(gamma), (NO, C))),
        "betB": np.ascontiguousarray(np.broadcast_to(f(beta), (NO, C))),
        "id128": np.eye(128, dtype=np.float32),
    }
    pr = f(pair)[0]
    tr = f(trans)[0] * SCALE
    ro = f(rot)[0].reshape(N, 9)
    in_maps = []
    for c in range(NCORES):
        lo, hi = c * NO, (c + 1) * NO
        m = dict(common)
        m["sTo"] = np.ascontiguousarray(s[lo:hi].T)
        m["so"] = np.ascontiguousarray(s[lo:hi])
        m["pairT"] = np.ascontiguousarray(pr[lo:hi].transpose(2, 0, 1))
        m["roto"] = np.ascontiguousarray(ro[lo:hi])
        m["transqo"] = np.ascontiguousarray(tr[lo:hi])
        in_maps.append(m)
    return in_maps


def run(in_maps, **kwargs):
    nc = _get_program()
    return run_bass_kernel_spmd(nc, in_maps, core_ids=list(range(NCORES)), **kwargs)


def kernel(**inputs):
    res = run(make_in_maps(**inputs))
    out = np.concatenate([res.results[c]["out"] for c in range(NCORES)], axis=0)
    return out.reshape(B, N, C).astype(np.float32)



# revision 8
# speedup vs baseline: 2.7150x; 2.7150x over previous
"""Invariant Point Attention kernel for Trainium2, 8-core SPMD.

Strategy: sequence-parallel over the query axis n (96 rows/core). Each core
computes full k/v/k_pts from `single` (replicated, tiny), its own q rows, its
own [96, 768] slice of the pair tensor, full attention rows (softmax over m is
core-local -> zero collectives), and its [96, 384] output slice.

v2: bf16 on the whole matmul datapath (fp32 weight loads + 4-cyc/row fp32
streaming dominated v1 at 461 us). pair is host-cast to bf16 and host-laid-out
as [PC, mt, n, mj] so each m-tile's slab is one contiguous DMA and each pair
matmul's stationary operand is a contiguous 128-col block (keeps FWL active).
The m-tile loop pipelines pair DMA -> pair-bias matmuls -> logits -> exp ->
attn@v accumulation. Denominators ride along as a ones-column in v_aug.

Math notes vs the reference:
  - terms constant along the softmax axis m cancel exactly (q2, bk, bpb) and
    are dropped;
  - SCALE is folded into Wq/bq, Wqp/bqp and the q-side trans on the host;
  - softmax runs without max-subtraction (logits are O(10), exp is safe in
    fp32); the denominator is applied after the attn@v matmul by linearity.
"""

import sys

for p in ("/opt/trn_rl_repo", "/opt/trn_rl_repo/concourse"):
    if p not in sys.path:
        sys.path.append(p)

import ml_dtypes
import numpy as np

import concourse.bass as bass
import concourse.tile as tile
from concourse import bacc, mybir
from concourse.bass_utils import run_bass_kernel_spmd

F32 = mybir.dt.float32
BF16 = mybir.dt.bfloat16
AX = mybir.AxisListType
ALU = mybir.AluOpType
ACTF = mybir.ActivationFunctionType

B, N, C, PC, H, P = 1, 768, 384, 128, 12, 3
Ch = C // H            # 32
HD = H * P * P         # 108
SCALE = Ch ** -0.5
EPS = 1e-5
NCORES = 8
NO = N // NCORES       # 96 own query rows per core
MT = N // 128          # 6 m tiles
VA = Ch + 1            # 33: v columns + denominator ones column


def _build(nc):
    def dt_(name, shape, dt=BF16):
        return nc.dram_tensor(name, shape, dt, kind="ExternalInput").ap()
    singleT = dt_("singleT", [C, N])
    sTo = dt_("sTo", [C, NO])
    so = dt_("so", [NO, C], F32)
    pair2 = dt_("pair2", [PC, MT, NO, 128])   # [pc, mt, n, mj]
    rot9 = dt_("rot9", [N, 9], F32)
    roto = dt_("roto", [NO, 9], F32)
    trans3 = dt_("trans3", [N, 3], F32)
    transqo = dt_("transqo", [NO, 3], F32)
    Wq = dt_("Wq", [C, 512])
    bq = dt_("bq", [1, 512])
    Wk = dt_("Wk", [C, 512])
    Wv = dt_("Wv", [C, C])
    bv = dt_("bv", [1, C])
    Wqp = dt_("Wqp", [C, HD])
    bqp = dt_("bqp", [1, HD])
    Wkp = dt_("Wkp", [C, HD])
    bkp = dt_("bkp", [1, HD])
    Wpb = dt_("Wpb", [PC, H])
    Wo = dt_("Wo", [C, C])
    bo = dt_("bo", [1, C])
    gamB = dt_("gamB", [NO, C], F32)
    betB = dt_("betB", [NO, C], F32)
    id128 = dt_("id128", [128, 128])
    out = nc.dram_tensor("out", [NO, C], F32, kind="ExternalOutput").ap()

    with tile.TileContext(nc) as tc:
        _kernel(tc, locals())
    return out


def _kernel(tc, t):
    nc = tc.nc
    mm = nc.tensor.matmul
    dma = nc.sync.dma_start

    const = tc.alloc_tile_pool(name="const", bufs=1)
    big = tc.alloc_tile_pool(name="big", bufs=1)

    # ---- load constants / weights (bf16 unless noted) ----
    def load(name, shape, src, dt=BF16):
        tl = const.tile(list(shape), dt, tag=name)
        dma(tl[:], src)
        return tl

    Wq_sb = const.tile([128, 3 * 512], BF16, tag="Wq_sb")
    Wk_sb = const.tile([128, 3 * 512], BF16, tag="Wk_sb")
    for W_sb, name in ((Wq_sb, "Wq"), (Wk_sb, "Wk")):
        for tt in range(3):
            dma(W_sb[:, tt * 512:(tt + 1) * 512], t[name][tt * 128:(tt + 1) * 128, :])
    Wv_sb = const.tile([128, 3 * C], BF16, tag="Wv_sb")
    Wo_sb = const.tile([128, 3 * C], BF16, tag="Wo_sb")
    for W_sb, name in ((Wv_sb, "Wv"), (Wo_sb, "Wo")):
        for tt in range(3):
            dma(W_sb[:, tt * C:(tt + 1) * C], t[name][tt * 128:(tt + 1) * 128, :])
    Wqp_sb = const.tile([128, 3 * HD], BF16, tag="Wqp_sb")
    Wkp_sb = const.tile([128, 3 * HD], BF16, tag="Wkp_sb")
    for W_sb, name in ((Wqp_sb, "Wqp"), (Wkp_sb, "Wkp")):
        for tt in range(3):
            dma(W_sb[:, tt * HD:(tt + 1) * HD], t[name][tt * 128:(tt + 1) * 128, :])
    Wpb_sb = load("Wpb_sb", (PC, H), t["Wpb"])
    bq_sb = load("bq_sb", (1, 512), t["bq"])
    bv_sb = load("bv_sb", (1, C), t["bv"])
    bqp_sb = load("bqp_sb", (1, HD), t["bqp"])
    bkp_sb = load("bkp_sb", (1, HD), t["bkp"])
    bo_sb = load("bo_sb", (1, C), t["bo"])
    id_sb = load("id_sb", (128, 128), t["id128"])
    gam_sb = load("gam_sb", (NO, C), t["gamB"], F32)
    bet_sb = load("bet_sb", (NO, C), t["betB"], F32)
    so_sb = load("so_sb", (NO, C), t["so"], F32)
    roto_sb = load("roto_sb", (NO, 9), t["roto"], F32)
    transqo_sb = load("transqo_sb", (NO, 3), t["transqo"], F32)
    sT_sb = const.tile([128, 3 * N], BF16, tag="sT_sb")
    for tt in range(3):
        dma(sT_sb[:, tt * N:(tt + 1) * N], t["singleT"][tt * 128:(tt + 1) * 128, :])
    sTo_sb = const.tile([128, 3 * NO], BF16, tag="sTo_sb")
    for tt in range(3):
        dma(sTo_sb[:, tt * NO:(tt + 1) * NO], t["sTo"][tt * 128:(tt + 1) * 128, :])
    rot_sb = const.tile([128, 6 * 9], F32, tag="rot_sb")
    trans_sb = const.tile([128, 6 * 3], F32, tag="trans_sb")
    for mt in range(MT):
        dma(rot_sb[:, mt * 9:(mt + 1) * 9], t["rot9"][mt * 128:(mt + 1) * 128, :])
        dma(trans_sb[:, mt * 3:(mt + 1) * 3], t["trans3"][mt * 128:(mt + 1) * 128, :])
    ones96 = const.tile([1, NO], BF16, tag="ones96")
    nc.vector.memset(ones96[:], 1.0)
    ones128 = const.tile([1, 128], BF16, tag="ones128")
    nc.vector.memset(ones128[:], 1.0)

    # ---- big persistent sbuf ----
    kT_sb = big.tile([128, 4 * N], BF16, tag="kT")        # [c_out, m] (3 row-tiles)
    qT_sb = big.tile([128, 4 * NO], BF16, tag="qT")       # [c_out, n]
    va_sb = big.tile([128, MT * H * VA], BF16, tag="va")  # per (mt, h): [v | ones]
    qg_sb = big.tile([NO, HD], BF16, tag="qg")
    kg_sb = big.tile([128, MT * HD], BF16, tag="kg")
    # per-head transposed points, head h at partitions 32*(h%3), col block h//3
    qgT_sb = big.tile([128, 4 * NO], BF16, tag="qgT")
    kgT_sb = big.tile([128, 4 * N], BF16, tag="kgT")
    k2s_sb = big.tile([128, MT * H], F32, tag="k2s")      # -0.5*SCALE*k2, per m-tile
    E_sb = big.tile([128, MT * H * NO], BF16, tag="E")    # exp(logits) [m | (mt, h, n)]
    nc.vector.memset(va_sb[:], 1.0)                       # ones columns pre-set

    with tc.tile_pool(name="pro", bufs=3, space="PSUM") as pro, \
         tc.tile_pool(name="work", bufs=6) as work:

        # ---- kT = (single @ Wk)^T : [c_out, m], no bias (cancels in softmax)
        for j in range(4):
            for half in range(2):
                ps = pro.tile([128, 512], F32, tag="ps")
                for tt in range(3):
                    mm(ps[:, :384], Wk_sb[:, tt * 512 + j * 128: tt * 512 + (j + 1) * 128],
                       sT_sb[:, tt * N + half * 384: tt * N + (half + 1) * 384],
                       start=(tt == 0), stop=(tt == 2))
                nc.vector.tensor_copy(kT_sb[:, j * N + half * 384: j * N + (half + 1) * 384], ps[:, :384])

        # ---- qT = (single_own @ (SCALE*Wq))^T + SCALE*bq : [c_out, n]
        for j in range(4):
            ps = pro.tile([128, 512], F32, tag="ps")
            for tt in range(3):
                mm(ps[:, :NO], Wq_sb[:, tt * 512 + j * 128: tt * 512 + (j + 1) * 128],
                   sTo_sb[:, tt * NO:(tt + 1) * NO], start=(tt == 0), stop=False)
            mm(ps[:, :NO], bq_sb[0:1, j * 128:(j + 1) * 128], ones96[:], start=False, stop=True)
            nc.vector.tensor_copy(qT_sb[:, j * NO:(j + 1) * NO], ps[:, :NO])

        # ---- v = single @ Wv + bv : [m, c_out] -> va_sb with ones gaps
        for mt in range(MT):
            ps = pro.tile([128, 512], F32, tag="ps")
            for tt in range(3):
                mm(ps[:, :384], sT_sb[:, tt * N + mt * 128: tt * N + (mt + 1) * 128],
                   Wv_sb[:, tt * C:(tt + 1) * C], start=(tt == 0), stop=False)
            mm(ps[:, :384], ones128[:], bv_sb[:], start=False, stop=True)
            dst = va_sb[:, mt * H * VA:(mt + 1) * H * VA] \
                .rearrange("p (h c) -> p h c", c=VA)[:, :, 0:Ch]
            nc.vector.tensor_copy(dst, ps[:, :384].rearrange("p (h c) -> p h c", c=Ch))

        # ---- point projections: qp [n, 108] (SCALE folded), kp per m-tile
        qp_sb = work.tile([NO, HD], BF16, tag="qp")
        ps = pro.tile([128, 512], F32, tag="ps")
        for tt in range(3):
            mm(ps[:NO, :HD], sTo_sb[:, tt * NO:(tt + 1) * NO],
               Wqp_sb[:, tt * HD:(tt + 1) * HD], start=(tt == 0), stop=False)
        mm(ps[:NO, :HD], ones96[:], bqp_sb[:], start=False, stop=True)
        nc.vector.tensor_copy(qp_sb[:], ps[:NO, :HD])

        kp_tiles = []
        for mt in range(MT):
            ps = pro.tile([128, 512], F32, tag="ps")
            for tt in range(3):
                mm(ps[:, :HD], sT_sb[:, tt * N + mt * 128: tt * N + (mt + 1) * 128],
                   Wkp_sb[:, tt * HD:(tt + 1) * HD], start=(tt == 0), stop=False)
            mm(ps[:, :HD], ones128[:], bkp_sb[:], start=False, stop=True)
            kp = work.tile([128, HD], BF16, tag="kp")
            nc.vector.tensor_copy(kp[:], ps[:, :HD])
            kp_tiles.append(kp)

        # ---- rotations: g[n,h,d,j] = sum_i p[n,h,d,i]*rot[n,3i+j] (+ trans[n,d])
        def rotate(dst, src, rsb, roff, tsb, toff, rows):
            dv = dst.rearrange("p (h d j) -> p h d j", d=3, j=3)
            sv = src.rearrange("p (h d i) -> p h d i", d=3, i=3)
            for j in range(3):
                acc = work.tile([rows, 36], F32, tag="rotacc")
                av = acc[:].rearrange("p (h d) -> p h d", d=3)
                nc.vector.tensor_scalar_mul(av, sv[:, :, :, 0], rsb[:rows, roff + j: roff + j + 1])
                for i in (1, 2):
                    nc.vector.scalar_tensor_tensor(
                        av, sv[:, :, :, i], rsb[:rows, roff + 3 * i + j: roff + 3 * i + j + 1],
                        av, op0=ALU.mult, op1=ALU.add)
                nc.vector.tensor_copy(dv[:, :, :, j], av)
            for d in range(3):
                nc.vector.tensor_scalar_add(dv[:, :, d, :], dv[:, :, d, :],
                                            tsb[:rows, toff + d: toff + d + 1])

        rotate(qg_sb[:], qp_sb[:], roto_sb, 0, transqo_sb, 0, NO)
        for mt in range(MT):
            rotate(kg_sb[:, mt * HD:(mt + 1) * HD], kp_tiles[mt][:],
                   rot_sb, mt * 9, trans_sb, mt * 3, 128)

        # ---- k2s = -0.5*SCALE*sum_dj kg^2 : [m, h] per m-tile
        for mt in range(MT):
            sq = work.tile([128, HD], F32, tag="sq")
            kgs = kg_sb[:, mt * HD:(mt + 1) * HD]
            nc.vector.tensor_mul(sq[:], kgs, kgs)
            red = work.tile([128, H], F32, tag="red")
            nc.vector.tensor_reduce(red[:], sq[:].rearrange("p (h e) -> p h e", e=9),
                                    axis=AX.X, op=ALU.add)
            nc.vector.tensor_scalar_mul(k2s_sb[:, mt * H:(mt + 1) * H], red[:], -0.5 * SCALE)

        # ---- transpose qg, kg -> per-head [(d,j)=9 rows @ 32*(h%3), n/m]
        ps = pro.tile([128, 512], F32, tag="ps")
        for h in range(H):
            bp, blk = 32 * (h % 3), h // 3
            mm(ps[bp:bp + 9, blk * NO:(blk + 1) * NO], qg_sb[:, h * 9:(h + 1) * 9],
               id_sb[:NO, :NO], start=True, stop=True)
        for bp in (0, 32, 64):
            nc.vector.tensor_copy(
                qgT_sb[bp:bp + 9].rearrange("p (blk n) -> p blk n", blk=4),
                ps[bp:bp + 9, :384].rearrange("p (blk n) -> p blk n", blk=4))
        for mt in range(MT):
            ps = pro.tile([128, 512], F32, tag="ps")
            for h in range(H):
                bp, blk = 32 * (h % 3), h // 3
                mm(ps[bp:bp + 9, blk * 128:(blk + 1) * 128],
                   kg_sb[:, mt * HD + h * 9: mt * HD + (h + 1) * 9],
                   id_sb[:], start=True, stop=True)
            for bp in (0, 32, 64):
                nc.vector.tensor_copy(
                    kgT_sb[bp:bp + 9].rearrange("p (blk m) -> p blk m", blk=4)
                    [:, :, mt * 128:(mt + 1) * 128],
                    ps[bp:bp + 9].rearrange("p (blk m) -> p blk m", blk=4))

    # ---- pipelined pair-bias + attention over m-tiles ----
    with tc.tile_pool(name="pgp", bufs=3) as pgp, \
         tc.tile_pool(name="ppair", bufs=3, space="PSUM") as ppair, \
         tc.tile_pool(name="pL", bufs=4, space="PSUM") as pL, \
         tc.tile_pool(name="pacc", bufs=1, space="PSUM") as pacc, \
         tc.tile_pool(name="att", bufs=2) as att:
        av_ps = pacc.tile([NO, H * VA], F32, tag="av")
        pg_tiles = []
        for mt in range(MT):
            pg = pgp.tile([128, NO * 128], BF16, tag="pg")
            dma(pg[:], t["pair2"][:, mt, :, :])
            pg_tiles.append(pg)
        for mt in range(MT):
            pg = pg_tiles[mt]
            # pair bias: per n, [pc,128m] x [pc,12h] -> [m, (n,h)] in 3 psum tiles
            pb = att.tile([128, H * NO], BF16, tag="pb")   # [m, (h, n)]
            for nb in range(3):
                psp = ppair.tile([128, 384], F32, tag="psp")
                for ns in range(32):
                    n = nb * 32 + ns
                    mm(psp[:, ns * H:(ns + 1) * H], pg[:, n * 128:(n + 1) * 128],
                       Wpb_sb[:], start=True, stop=True)
                dst = pb[:].rearrange("p (h n) -> p h n", h=H)[:, :, nb * 32:(nb + 1) * 32] \
                    .transpose([0, 2, 1])
                nc.vector.tensor_copy(dst, psp[:].rearrange("p (n h) -> p n h", h=H))
            # logits: qk + point terms; one PSUM tile per accumulation group
            # (multi-mm groups sharing a PSUM bank hang TRN2)
            tmp = att.tile([128, H * NO], F32, tag="tmp")
            for h in range(H):
                Lp = pL.tile([128, NO], F32, tag="L")
                tl, tr = h // 3, 32 * (h % 3)
                mm(Lp[:],
                   kT_sb[tr:tr + 32, tl * N + mt * 128: tl * N + (mt + 1) * 128],
                   qT_sb[tr:tr + 32, tl * NO:(tl + 1) * NO], start=True, stop=False)
                mm(Lp[:],
                   kgT_sb[tr:tr + 9, tl * N + mt * 128: tl * N + (mt + 1) * 128],
                   qgT_sb[tr:tr + 9, tl * NO:(tl + 1) * NO], start=False, stop=True)
                nc.vector.scalar_tensor_tensor(
                    tmp[:, h * NO:(h + 1) * NO], Lp[:],
                    k2s_sb[:, mt * H + h: mt * H + h + 1],
                    pb[:, h * NO:(h + 1) * NO], op0=ALU.add, op1=ALU.add)
            nc.scalar.activation(E_sb[:, mt * H * NO:(mt + 1) * H * NO], tmp[:], ACTF.Exp)

        # ---- attn@v (+ denominator via ones column), accumulate over mt ----
        for h in range(H):
            for mt in range(MT):
                mm(av_ps[:, h * VA:(h + 1) * VA],
                   E_sb[:, (mt * H + h) * NO:(mt * H + h + 1) * NO],
                   va_sb[:, (mt * H + h) * VA:(mt * H + h + 1) * VA],
                   start=(mt == 0), stop=(mt == MT - 1))

        # ---- epilogue: divide, out-proj, residual, layernorm ----
        with tc.tile_pool(name="epi", bufs=1) as epi:
            avv = av_ps[:].rearrange("p (h c) -> p h c", c=VA)
            rcp = epi.tile([NO, H], F32, tag="rcp")
            nc.vector.reciprocal(rcp[:], avv[:, :, Ch])
            w_sb = epi.tile([NO, C], BF16, tag="w")
            for h in range(H):
                nc.vector.tensor_scalar_mul(w_sb[:, h * Ch:(h + 1) * Ch],
                                            avv[:, h, 0:Ch], rcp[:, h:h + 1])
            wT_sb = epi.tile([128, 3 * NO], BF16, tag="wT")
            for tt in range(3):
                tp = pL.tile([128, NO], F32, tag="L")
                mm(tp[:], w_sb[:, tt * 128:(tt + 1) * 128], id_sb[:NO, :NO],
                   start=True, stop=True)
                nc.vector.tensor_copy(wT_sb[:, tt * NO:(tt + 1) * NO], tp[:])
            o_ps = pacc.tile([NO, H * VA], F32, tag="av")
            for tt in range(3):
                mm(o_ps[:, :C], wT_sb[:, tt * NO:(tt + 1) * NO], Wo_sb[:, tt * C:(tt + 1) * C],
                   start=(tt == 0), stop=False)
            mm(o_ps[:, :C], ones96[:], bo_sb[:], start=False, stop=True)
            x_sb = epi.tile([NO, C], F32, tag="x")
            nc.vector.tensor_add(x_sb[:], o_ps[:, :C], so_sb[:])
            mu = epi.tile([NO, 1], F32, tag="mu")
            nc.vector.tensor_reduce(mu[:], x_sb[:], axis=AX.X, op=ALU.add)
            nc.vector.tensor_scalar_mul(mu[:], mu[:], 1.0 / C)
            xm = epi.tile([NO, C], F32, tag="xm")
            nc.vector.tensor_scalar_sub(xm[:], x_sb[:], mu[:])
            sq = epi.tile([NO, C], F32, tag="sqe")
            nc.vector.tensor_mul(sq[:], xm[:], xm[:])
            var = epi.tile([NO, 1], F32, tag="var")
            nc.vector.tensor_reduce(var[:], sq[:], axis=AX.X, op=ALU.add)
            epsb = epi.tile([NO, 1], F32, tag="epsb")
            nc.vector.memset(epsb[:], EPS)
            std = epi.tile([NO, 1], F32, tag="std")
            nc.scalar.activation(std[:], var[:], ACTF.Sqrt, bias=epsb[:], scale=1.0 / C)
            rstd = epi.tile([NO, 1], F32, tag="rstd")
            nc.vector.reciprocal(rstd[:], std[:])
            y = epi.tile([NO, C], F32, tag="y")
            nc.vector.tensor_scalar_mul(y[:], xm[:], rstd[:])
            nc.vector.tensor_mul(y[:], y[:], gam_sb[:])
            nc.vector.tensor_add(y[:], y[:], bet_sb[:])
            dma(t["out"], y[:])
    big.release()
    const.release()


_CACHE = {}


def _get_program():
    if "nc" not in _CACHE:
        nc = bacc.Bacc("TRN2", target_bir_lowering=False, debug=False,
                       num_devices=NCORES)
        _build(nc)
        nc.compile()
        _CACHE["nc"] = nc
    return _CACHE["nc"]


def _pad_heads(W):
    # scatter head h (32 cols) to col 128*(h//3) + 32*(h%3) of a 512-wide buffer
    out = np.zeros(W.shape[:-1] + (512,), np.float32)
    for h in range(H):
        out[..., 128 * (h // 3) + 32 * (h % 3): 128 * (h // 3) + 32 * (h % 3) + Ch] = \
            W[..., h * Ch:(h + 1) * Ch]
    return out


def make_in_maps(single, pair, rot, trans, Wq, bq, Wk, bk, Wv, bv, Wpb, bpb,
                 Wqp, bqp, Wkp, bkp, Wo, bo, gamma, beta):
    f = lambda a: np.ascontiguousarray(np.asarray(a), dtype=np.float32)
    bf = lambda a: np.ascontiguousarray(np.asarray(a, dtype=np.float32).astype(ml_dtypes.bfloat16))
    s = f(single)[0]
    common = {
        "singleT": bf(s.T),
        "rot9": f(rot)[0].reshape(N, 9),
        "trans3": f(trans)[0],
        "Wq": bf(_pad_heads(f(Wq) * SCALE)), "bq": bf(_pad_heads((f(bq) * SCALE).reshape(1, C))),
        "Wk": bf(_pad_heads(f(Wk))),
        "Wv": bf(Wv), "bv": bf(f(bv).reshape(1, C)),
        "Wqp": bf(f(Wqp) * SCALE), "bqp": bf((f(bqp) * SCALE).reshape(1, HD)),
        "Wkp": bf(Wkp), "bkp": bf(f(bkp).reshape(1, HD)),
        "Wpb": bf(Wpb),
        "Wo": bf(Wo), "bo": bf(f(bo).reshape(1, C)),
        "gamB": np.ascontiguousarray(np.broadcast_to(f(gamma), (NO, C))),
        "betB": np.ascontiguousarray(np.broadcast_to(f(beta), (NO, C))),
        "id128": np.eye(128, dtype=ml_dtypes.bfloat16),
    }
    pr = np.asarray(pair)[0]
    tr = f(trans)[0] * SCALE
    ro = f(rot)[0].reshape(N, 9)
    in_maps = []
    for c in range(NCORES):
        lo, hi = c * NO, (c + 1) * NO
        m = dict(common)
        m["sTo"] = bf(s[lo:hi].T)
        m["so"] = np.ascontiguousarray(s[lo:hi])
        # [n, m, pc] -> [pc, mt, n, mj]
        m["pair2"] = np.ascontiguousarray(
            np.asarray(pr[lo:hi], dtype=ml_dtypes.bfloat16)
            .reshape(NO, MT, 128, PC).transpose(3, 1, 0, 2))
        m["roto"] = np.ascontiguousarray(ro[lo:hi])
        m["transqo"] = np.ascontiguousarray(tr[lo:hi])
        in_maps.append(m)
    return in_maps


def run(in_maps, **kwargs):
    nc = _get_program()
    return run_bass_kernel_spmd(nc, in_maps, core_ids=list(range(NCORES)), **kwargs)


def kernel(**inputs):
    res = run(make_in_maps(**inputs))
    out = np.concatenate([res.results[c]["out"] for c in range(NCORES)], axis=0)
    return out.reshape(B, N, C).astype(np.float32)
